# revision 1
# baseline (speedup 1.0000x reference)
"""Trainium2 Bass kernel for relational GNN message passing (SpMM).

Computes: out = weight[idx] * segment_sum(edge_vals[idx][:,None] * x[edge_cols[idx]],
                                          edge_rows[idx], N)

Strategy (8 NeuronCores, SPMD — one program, per-core data):
- Host: sort edges by destination row; shard destination rows across the 8
  cores (ceil(N/8) rows each); within a core, bucket edges by 128-row
  destination block; split each bucket by source half (dma_gather indices are
  int16, so sources are addressed as x_lo = x[:N/2], x_hi = x[N/2:]); pad each
  (block, half) bucket to a multiple of 128 edges.  The chunk schedule (chunks
  per block/half) is the max over cores so all cores run the same program.
- Device (per core): batched dma_gather pulls 256B source rows into SBUF, one
  edge per partition (128 edges per chunk).  For each chunk the DVE builds a
  selection matrix sel[e, d] = vals[e] * (rows_rel[e] == d) with one
  tensor_scalar(iota, is_equal, rows_rel, mult, vals).  The tensor engine
  accumulates psum[64 feat, 128 dst] += xs[128e, 64].T @ sel[128e, 128] over a
  block's chunks.  Eviction multiplies by weight[idx] (ACT engine, per-
  partition scale) into an SBUF stage, DMA'd out as out_t[64, n_rows_padded].
- Host: transpose each core's out_t and concatenate.
"""

import sys

for _p in ("/opt/trn_rl_repo",):
    if _p not in sys.path:
        sys.path.insert(0, _p)

from contextlib import ExitStack

import numpy as np

from concourse import bacc, mybir, tile
from concourse.bass_utils import run_bass_kernel_spmd

P = 128           # partitions / edges per chunk / dst rows per block
NCORES = 8
GB = 16           # gather batch: chunks per dma_gather call

# Set by test.py to capture an NTFF profile; harness leaves these alone.
TRACE = False
TRACE_DIR = None
LAST_EXEC_NS = None

_PROGRAM_CACHE = {}


def _build_program(D, n_lo_src, n_hi_src, M, out_cols):
    """Build the SPMD Bass program for chunk schedule M[NBLK, 2]."""
    NBLK = M.shape[0]
    C_lo = int(M[:, 0].sum())
    C_hi = int(M[:, 1].sum())

    nc = bacc.Bacc("TRN2", target_bir_lowering=False, debug=False,
                   num_devices=NCORES, num_swdge_queues=4)

    x_lo_d = nc.dram_tensor("x_lo", [n_lo_src, D], mybir.dt.float32,
                            kind="ExternalInput")
    x_hi_d = nc.dram_tensor("x_hi", [n_hi_src, D], mybir.dt.float32,
                            kind="ExternalInput")
    idx_d = {}
    meta_d = {}
    for s, C in (("lo", C_lo), ("hi", C_hi)):
        if C:
            idx_d[s] = nc.dram_tensor(f"idx_{s}", [P, C * 8], mybir.dt.int16,
                                      kind="ExternalInput")
            meta_d[s] = nc.dram_tensor(f"meta_{s}", [P, C * 2],
                                       mybir.dt.float32, kind="ExternalInput")
    iota_d = nc.dram_tensor("iota", [P, P], mybir.dt.float32,
                            kind="ExternalInput")
    w_d = nc.dram_tensor("w", [P, 1], mybir.dt.float32, kind="ExternalInput")
    out_d = nc.dram_tensor("out_t", [D, out_cols], mybir.dt.float32,
                           kind="ExternalOutput")

    x_src = {"lo": x_lo_d, "hi": x_hi_d}
    C_stream = {"lo": C_lo, "hi": C_hi}

    with tile.TileContext(nc) as tc, ExitStack() as ctx:
        const = ctx.enter_context(tc.tile_pool(name="const", bufs=1))
        xs_pool = ctx.enter_context(tc.tile_pool(name="xs", bufs=12))
        selp = ctx.enter_context(tc.tile_pool(name="sel", bufs=8))
        psum = ctx.enter_context(tc.tile_pool(name="psum", bufs=4,
                                              space="PSUM"))
        psc = ctx.enter_context(tc.tile_pool(name="psc", bufs=1, space="PSUM"))
        outp = ctx.enter_context(tc.tile_pool(name="outp", bufs=1))

        iota_t = const.tile([P, P], mybir.dt.float32, tag="iota")
        nc.sync.dma_start(out=iota_t[:], in_=iota_d[:])
        iota_ps = psc.tile([P, P], mybir.dt.float32, space="PSUM", tag="iops")
        nc.vector.tensor_copy(out=iota_ps[:], in_=iota_t[:])
        w_t = const.tile([P, 1], mybir.dt.float32, tag="w")
        nc.sync.dma_start(out=w_t[:], in_=w_d[:])

        idx_t = {}
        meta_t = {}
        for s in ("lo", "hi"):
            C = C_stream[s]
            if not C:
                continue
            idx_t[s] = const.tile([P, C * 8], mybir.dt.int16, tag=f"idx{s}", name=f"idx_t_{s}")
            nc.sync.dma_start(out=idx_t[s][:], in_=idx_d[s][:])
            meta_t[s] = const.tile([P, C, 2], mybir.dt.float32, tag=f"meta{s}", name=f"meta_t_{s}")
            nc.sync.dma_start(
                out=meta_t[s][:],
                in_=meta_d[s][:].rearrange("p (c k) -> p c k", k=2),
            )

        stage = outp.tile([P, out_cols], mybir.dt.float32, tag="stage")

        xs_tiles = {}  # (stream, batch) -> tile
        gcount = [0]

        def get_xs(s, pos):
            g = pos // GB
            if (s, g) not in xs_tiles:
                lo = g * GB
                hi = min(lo + GB, C_stream[s])
                rem = hi - lo
                t = xs_pool.tile([P, GB, D], mybir.dt.float32, tag="xs", name=f"xs_{s}_{g}")
                nc.gpsimd.dma_gather(
                    t[:, :rem, :],
                    x_src[s][:],
                    idx_t[s][:, lo * 8 : hi * 8],
                    rem * P,
                    rem * P,
                    D,
                    single_packet=False,
                    queue_num=gcount[0] % 4,
                )
                gcount[0] += 1
                xs_tiles[(s, g)] = t
            return xs_tiles[(s, g)], pos % GB

        ptr = {"lo": 0, "hi": 0}
        for b in range(NBLK):
            nch = int(M[b, 0] + M[b, 1])
            ps = psum.tile([D, P], mybir.dt.float32, space="PSUM", tag="ps")
            k = 0
            for s, m in (("lo", int(M[b, 0])), ("hi", int(M[b, 1]))):
                for _ in range(m):
                    pos = ptr[s]
                    ptr[s] += 1
                    xs_t, slot = get_xs(s, pos)
                    sel = selp.tile([P, P], mybir.dt.float32, tag="sel")
                    nc.vector.tensor_scalar(
                        out=sel[:],
                        in0=iota_ps[:],
                        scalar1=meta_t[s][:, pos, 0:1],
                        scalar2=meta_t[s][:, pos, 1:2],
                        op0=mybir.AluOpType.is_equal,
                        op1=mybir.AluOpType.mult,
                    )
                    nc.tensor.matmul(
                        ps[:],
                        lhsT=xs_t[:, slot, :],
                        rhs=sel[:],
                        start=(k == 0),
                        stop=(k == nch - 1),
                    )
                    k += 1
            nc.scalar.activation(
                out=stage[:D, b * P : (b + 1) * P],
                in_=ps[:],
                func=mybir.ActivationFunctionType.Copy,
                scale=w_t[:D, 0:1],
            )
        nc.sync.dma_start(out=out_d[:], in_=stage[:D, :])

    nc.compile()
    return nc


def kernel(x, weight, edge_vals, edge_rows, edge_cols, idx):
    global LAST_EXEC_NS

    x = np.ascontiguousarray(np.asarray(x, dtype=np.float32))
    weight = np.asarray(weight, dtype=np.float32)
    i = int(np.asarray(idx))
    rows = np.asarray(edge_rows[i], dtype=np.int64)
    cols = np.asarray(edge_cols[i], dtype=np.int64)
    vals = np.asarray(edge_vals[i], dtype=np.float32)

    N, D = x.shape
    E = rows.shape[0]
    assert D == 64, D
    RPC = -(-N // NCORES)          # dst rows per core
    NBLK = -(-RPC // P)            # dst blocks per core
    XH = -(-N // 2)                # source half size
    assert XH <= 32768, XH
    n_hi_src = N - XH

    # ---- host prep: group edges by (core, block, half) -------------------
    core = rows // RPC
    rel = rows - core * RPC
    block = rel >> 7
    half = (cols >= XH).astype(np.int64)
    ngrp = NCORES * NBLK * 2
    key = (core * NBLK + block) * 2 + half
    order = np.argsort(key, kind="stable")
    ks = key[order]
    cnt = np.bincount(ks, minlength=ngrp)
    starts = np.concatenate(([0], np.cumsum(cnt)[:-1]))
    within = np.arange(E, dtype=np.int64) - starts[ks]

    # chunk schedule: max over cores, shared by the SPMD program
    cnt_cbh = cnt.reshape(NCORES, NBLK, 2)
    M = -(-cnt_cbh.max(axis=0) // P)           # [NBLK, 2]
    M[(M.sum(axis=1) == 0), 0] = 1             # empty block -> one pad chunk
    C_lo = int(M[:, 0].sum())
    C_hi = int(M[:, 1].sum())
    off_lo = np.concatenate(([0], np.cumsum(M[:, 0])[:-1])) * P
    off_hi = np.concatenate(([0], np.cumsum(M[:, 1])[:-1])) * P
    # slot offset per group g=(c,b,h)
    slot_off = np.empty((NBLK, 2), np.int64)
    slot_off[:, 0] = off_lo
    slot_off[:, 1] = off_hi
    slot_off_g = np.tile(slot_off.reshape(-1), NCORES)
    slots = slot_off_g[ks] + within

    rows_rel_s = (rel[order] & 127).astype(np.float32)
    colh_s = (cols[order] - XH * half[order]).astype(np.int16)
    vals_s = vals[order]
    core_s = ks // (NBLK * 2)
    half_s = ks & 1

    iota = np.tile(np.arange(P, dtype=np.float32), (P, 1))
    wvec = np.full((P, 1), weight[i], np.float32)
    out_cols = NBLK * P

    def pack_idx(flat16, C):
        # logical slot j -> [j % 16, j // 16], replicated across 8 Q7 cores
        a = flat16.reshape(C * 8, 16).T
        return np.tile(a, (NCORES, 1))

    def pack_meta(rows_rel_f, vals_f, C):
        m = np.stack([rows_rel_f.reshape(C, P), vals_f.reshape(C, P)],
                     axis=-1)
        return np.ascontiguousarray(m.transpose(1, 0, 2)).reshape(P, C * 2)

    in_maps = []
    for c in range(NCORES):
        m = core_s == c
        sl = slots[m]
        hs = half_s[m]
        im = {
            "x_lo": x[:XH],
            "x_hi": x[XH:],
            "iota": iota,
            "w": wvec,
        }
        for s, C, hbit in (("lo", C_lo, 0), ("hi", C_hi, 1)):
            if not C:
                continue
            i16 = np.zeros(C * P, np.int16)
            rr = np.zeros(C * P, np.float32)
            vv = np.zeros(C * P, np.float32)
            mh = m.copy()
            mh[m] = hs == hbit
            sh = slots[mh]
            i16[sh] = colh_s[mh]
            rr[sh] = rows_rel_s[mh]
            vv[sh] = vals_s[mh]
            im[f"idx_{s}"] = pack_idx(i16, C)
            im[f"meta_{s}"] = pack_meta(rr, vv, C)
        in_maps.append(im)

    # ---- build / fetch program ------------------------------------------
    sig = (D, XH, n_hi_src, out_cols, M.tobytes())
    if sig not in _PROGRAM_CACHE:
        _PROGRAM_CACHE[sig] = _build_program(D, XH, n_hi_src, M, out_cols)
    nc = _PROGRAM_CACHE[sig]

    kw = {}
    if TRACE:
        kw = dict(trace=True, tmpdir=TRACE_DIR)
    res = run_bass_kernel_spmd(nc, in_maps, list(range(NCORES)), **kw)
    LAST_EXEC_NS = res.exec_time_ns

    out = np.empty((N, D), np.float32)
    for c in range(NCORES):
        lo = c * RPC
        hi = min(lo + RPC, N)
        out[lo:hi] = res.results[c]["out_t"].T[: hi - lo]
    return out

